# revision 31
# baseline (speedup 1.0000x reference)
"""Balanced CE loss on 8 Trainium2 NeuronCores — raw Bass (hand-synced).

Math: z = t ? p*p : (1-p); loss = -mean(ln z)   (ln(p^2) == 2 ln p, w1=2, w0=1)

Engine split per chunk i:
  Sync  : dma p/t groups (see DMA_PLAN) -> +16 pl/tl lane sems
  ACT   : first p-group dma on its own HWDGE ring (parallel descriptor gen
          with Sync at startup), then LN_i ln(z) + accum col i -> +1 s_ln
  DVE   : Z_i   z = t ? p*p : 1-p   -> +1 s_z      (one fused custom DVE op,
          registered in dve_ops.OPS at import — the documented per-NEFF
          custom-op path; select cond is the raw int32 t, converted at the
          DVE read port)

With the select fused into one DVE instruction the per-chunk engine cost is
DVE ~2.3us + ACT ~2.3us against a ~4.9us DMA slot (2 MiB @ ~420 GB/s), so
the kernel is DMA-bound with ~2x compute slack instead of the knife-edge
balance of the 4-op version (SQ+LN on ACT, OM+CP on DVE ~= 5.0/4.6us/slot).
Measured fast-path structure: ~6.9us NEFF preamble (fixed rendezvous +
table DMAs) + ~0.8us HWDGE start + ~79.9us gapless input stream at
~420 GB/s + ~5us tail (last-chunk Z+LN+RA, store receipt, postamble).

DMA completion uses round-robin lane sems (a single sem cannot prove a given
tile landed when several DMAs are in flight: their 16 per-engine increments
interleave).  NRT does not reset semaphores between invocations, so ours are
cleared at the END of the kernel (past the Block-exit barrier); each run then
starts from zero with no start-of-kernel fence, letting the first DMA issue
as soon as the Sync engine boots.  The tail chunks taper so the
post-last-DMA compute chain is short.
"""

import numpy as np

import concourse.bacc as bacc
import concourse.bass as bass
import concourse.dve_ops as dve_ops
import concourse.mybir as mybir
from concourse.bass_utils import run_bass_kernel_spmd
from concourse.dve_spec import Spec, Src0, Src1, One, select, sq, lower, _has_src1
from concourse.dve_uop import DveOpSpec

N = 33554432
NCORES = 8
NSHARD = N // NCORES  # 4194304
P = 128
M = NSHARD // P  # 32768 f32 per partition

F = 2048  # slot width (max chunk width)
# Gradual tail taper: each tail chunk's Z+LN+READ_ACC must finish inside the
# stream time of the chunks after it (~2.47ns per unit width for the p+t
# pair at ~420 GB/s), or ACT work piles up serially after the stream ends.
# Searched numerically against the measured cost model; only the last
# chunk's Z+LN+RA+store chain stays exposed.
CHUNKS = [2048] * 14 + [1408, 1024, 768, 512, 384]
assert sum(CHUNKS) == M
NT = len(CHUNKS)

# DMA plan per stream: 2-chunk (2 MiB) groups at the head (compute unlocks
# early enough to pace slot recycling), 4-chunk (4 MiB) groups mid-stream
# (fewer transfer boundaries, closer to the large-transfer rate), 2-chunk to
# land before the taper. The tail chunks stay one DMA each so their
# completions arrive spread out for the taper overlap.
DMA_PLAN = [(0, 2), (2, 2), (4, 4), (8, 4), (12, 2)] + [
    (i, 1) for i in range(14, NT)
]

# Compute plan: the 14 full-width chunks are processed as 7 pairs — one
# 4096-wide Z and one 4096-wide LN+accum per pair. Halving the instruction
# count halves the fixed per-op cost (DVE 58cyc, ACT 352cyc + 280ns
# READ_ACCUMULATOR): at nominal clock ACT drops from 2x(2000+280)=4560ns to
# 3986ns per pair against a ~10us pair arrival slot, and under the ~20%
# DVFS-throttled mode (observed) ACT stays comfortably ahead of the stream
# instead of accumulating a ~4us backlog. Pair slots are contiguous in
# pbuf/tbuf (KP=8, even starts) and zbuf (KZ=4); a [P,4096] f32 LN dump is
# exactly one PSUM's worth.
# One compute group per chunk. Paired (4096-wide) Z/LN groups were tried —
# they halve per-op fixed costs and help the DVFS-throttled mode — but a
# paired op near the taper is a ~4.3+4.0us serial block whose late start
# (gated by its merged DMA's completion) displaces the tail LNs past the
# stream end, and the fast-path samples regressed ~0.7us. Individual ops
# measured best (92.0us).
COMPUTE_PLAN = [(i, 1) for i in range(NT)]
NG = len(COMPUTE_PLAN)  # one acc column (partial sum) per compute group

_DMA_OF = {}
for _d, (_a, _n) in enumerate(DMA_PLAN):
    for _i in range(_a, _a + _n):
        _DMA_OF[_i] = _d


def dma_of(i):  # index of the DMA (per stream) that carries chunk i
    return _DMA_OF[i]


KP = 8  # p-tile slots (8 so 2-chunk DMAs at even chunk indices never wrap)
KT = 8  # t-tile slots
KZ = 4  # z slots
# 8 completion lanes per stream: a DMA's issue only waits for the DMA 8 back
# on its lane (one outstanding use per lane keeps the 16-inc count proof
# exact). With 4 lanes the ~600ns HWDGE issues couldn't run ahead through
# the 12 small tail DMAs and the SDMA engines idled ~2.4us mid-stream.
NL = 8

# merged DMAs land in consecutive F-wide slots, so they may only cover
# full-width chunks that don't wrap the slot rings
for _a, _n in DMA_PLAN:
    if _n > 1:
        assert (_a % KP) + _n <= KP and (_a % KT) + _n <= KT
        assert all(c == F for c in CHUNKS[_a : _a + _n])

WEIGHT0 = 1.0
WEIGHT1 = 2.0

_cache = {}

AF = mybir.ActivationFunctionType
ALU = mybir.AluOpType


def _register_bce_z():
    """z = select(t, p*p, 1-p) as one DVE instruction. Appended to
    dve_ops.OPS (the documented custom-op extension point); the uops sha is
    computed from lower() at import so the pin is self-consistent."""
    name = "BCE_Z_ANT"
    for op in dve_ops.OPS:
        if op.name == name:
            return op
    spec = Spec(
        body=select(Src1, sq(Src0), One - Src0),
        reference=lambda in0, in1: np.where(in1 != 0, in0 * in0, 1.0 - in0),
    )
    row = dve_ops._CUSTOM_DVE_ROW_BASE + len(dve_ops.OPS)
    shas = {
        ver: DveOpSpec(
            name=name, opcode=row, uops=lower(spec, ver=ver), rd1_en=_has_src1(spec)
        ).sha(ver)
        for ver in ("v3", "v4")
    }
    op = dve_ops.DveOp(name, spec, subdim=False, uops_sha=shas)
    dve_ops.OPS.append(op)
    dve_ops.CUSTOM_DVE_SPECS[name] = spec
    dve_ops._SUB_OPCODE_FOR_NAME[name] = row
    return op


BCE_Z = _register_bce_z()


def build_nc():
    # Bass.__init__ ends with an all_engine_barrier after the const memsets.
    # That barrier makes every engine wait for the slowest-booting one (~3.4us
    # measured, dominated by the unused PE/Tensor engine) before any DMA can
    # issue. Skip it; the only ordering it provided that this kernel needs is
    # const-memsets (GpSimd) vs ACT's bias read, covered by the s_const
    # handshake below.
    _orig_barrier = bass.Bass.all_engine_barrier
    bass.Bass.all_engine_barrier = lambda self, *a, **k: None
    try:
        nc = bacc.Bacc(
            "TRN2", target_bir_lowering=False, debug=False, num_devices=NCORES
        )
    finally:
        bass.Bass.all_engine_barrier = _orig_barrier

    x = nc.dram_tensor("input", [NSHARD], mybir.dt.float32, kind="ExternalInput").ap()
    t = nc.dram_tensor("target", [NSHARD], mybir.dt.int32, kind="ExternalInput").ap()
    out = nc.dram_tensor("out", [P, NG], mybir.dt.float32, kind="ExternalOutput").ap()

    xt = x.rearrange("(p m) -> p m", p=P)
    tt = t.rearrange("(p m) -> p m", p=P)

    offs = []
    o = 0
    for w in CHUNKS:
        offs.append(o)
        o += w

    pl = [nc.alloc_semaphore(f"s_p{j}") for j in range(NL)]
    tl = [nc.alloc_semaphore(f"s_t{j}") for j in range(NL)]
    s_out = nc.alloc_semaphore("s_out")
    s_z = nc.alloc_semaphore("s_z")
    s_ln = nc.alloc_semaphore("s_ln")
    s_const = nc.alloc_semaphore("s_const")
    sems = pl + tl + [s_out, s_z, s_ln, s_const]

    # Sems start at 0: zeroed by NRT at model load, and re-zeroed by OUR
    # end-of-kernel clears (after the Block-exit barrier) on every run. So no
    # start-of-kernel fence is needed and the first DMA can issue as soon as
    # the Sync engine boots. The only start-time ordering needed is the
    # framework's const memsets (GpSimd) vs ACT's bias read: a one-sem
    # handshake below covers it.

    with (
        nc.sbuf_tensor([P, KP * F], mybir.dt.float32) as pbuf,
        nc.sbuf_tensor([P, KT * F], mybir.dt.int32) as tbuf,
        nc.sbuf_tensor([P, KZ * F], mybir.dt.float32) as zbuf,
        nc.sbuf_tensor([P, NG], mybir.dt.float32) as acc,
        nc.sbuf_tensor([P, 1], mybir.dt.float32) as dummy,
        nc.psum_tensor([P, 2 * F], mybir.dt.float32) as lnall,
        nc.Block(no_gpsimd_drain=True) as block,
    ):

        # GpSimd ran the framework's const memsets in its preamble; publish
        # their completion for ACT (which reads the const-0.0 bias AP).
        @block.gpsimd
        def _(gp):
            gp.memset(dummy[:, :], 0.0).then_inc(s_const)

        # ---- Sync: DMA issue, paced by slot-free sems --------------------
        # The first p-group goes out on the Scalar engine's own HWDGE ring
        # (see the scalar block) concurrently with Sync's first t-group —
        # the two descriptor generators run in parallel, starting the
        # stream ~0.7us earlier.
        @block.sync
        def _(sync):
            for d, (a, n) in enumerate(DMA_PLAN):
                b = a + n - 1  # last chunk this DMA carries
                wsum = sum(CHUNKS[a : b + 1])
                if d > 0:
                    if b >= KP:
                        sync.wait_ge(s_z, b - KP + 1)
                    if d >= NL:
                        sync.wait_ge(pl[d % NL], 16 * (d // NL))
                    sync.dma_start(
                        out=pbuf[:, (a % KP) * F : (a % KP) * F + wsum],
                        in_=xt[:, offs[a] : offs[a] + wsum],
                    ).then_inc(pl[d % NL], 16)
                if b >= KT:
                    sync.wait_ge(s_z, b - KT + 1)
                if d >= NL:
                    sync.wait_ge(tl[d % NL], 16 * (d // NL))
                sync.dma_start(
                    out=tbuf[:, (a % KT) * F : (a % KT) * F + wsum],
                    in_=tt[:, offs[a] : offs[a] + wsum],
                ).then_inc(tl[d % NL], 16)
            sync.wait_ge(s_ln, NT)
            sync.dma_start(out=out[:], in_=acc[:]).then_inc(s_out, 16)
            # No completion wait here: the end-of-kernel dma_reset over our
            # sem range (emitted after the Block-exit barrier) drains this
            # DMA, so its receipt overlaps the barrier instead of preceding
            # it.

        # ---- DVE: z = t ? p*p : 1-p (one fused op per compute group) -----
        # s_z / s_ln stay in CHUNK units (group ops inc by the group size),
        # so the Sync slot-pacing waits are unchanged.
        @block.vector
        def _(vector):
            for a, n in COMPUTE_PLAN:
                b = a + n - 1
                w = sum(CHUNKS[a : b + 1])
                if b >= KZ:
                    vector.wait_ge(s_ln, b - KZ + 1)
                for d in sorted({dma_of(i) for i in range(a, b + 1)}):
                    vector.wait_ge(pl[d % NL], 16 * (d // NL + 1))
                    vector.wait_ge(tl[d % NL], 16 * (d // NL + 1))
                vector._custom_dve(
                    BCE_Z,
                    out=zbuf[:, (a % KZ) * F : (a % KZ) * F + w],
                    in0=pbuf[:, (a % KP) * F : (a % KP) * F + w],
                    in1=tbuf[:, (a % KT) * F : (a % KT) * F + w],
                ).then_inc(s_z, n)

        # ---- ACT: first p-group DMA on its own HWDGE ring, then ln(z)
        # with accum into acc column g (PSUM dump is same-engine in-order,
        # so the single lnall buffer needs no WAW sems) ---------------------
        @block.scalar
        def _(scalar):
            a0, n0 = DMA_PLAN[0]
            w0 = sum(CHUNKS[a0 : a0 + n0])
            scalar.dma_start(
                out=pbuf[:, (a0 % KP) * F : (a0 % KP) * F + w0],
                in_=xt[:, offs[a0] : offs[a0] + w0],
            ).then_inc(pl[0], 16)
            scalar.wait_ge(s_const, 1)
            for g, (a, n) in enumerate(COMPUTE_PLAN):
                b = a + n - 1
                w = sum(CHUNKS[a : b + 1])
                scalar.wait_ge(s_z, b + 1)
                scalar.activation(
                    lnall[:, :w],
                    zbuf[:, (a % KZ) * F : (a % KZ) * F + w],
                    AF.Ln,
                    accum_out=acc[:, g : g + 1],
                ).then_inc(s_ln, n)

    # Past the Block-exit barrier every engine is done: reset our sems (and
    # the DMA state tied to them, draining the in-flight output store) so the
    # next invocation starts from zero.
    for r in bass.compact_to_ranges([s.num for s in sems]):
        nc.gpsimd.dma_reset(r)
        nc.gpsimd.sem_clear(r)

    nc.compile()
    return nc


def kernel(input, target):
    if "nc" not in _cache:
        _cache["nc"] = build_nc()
    nc = _cache["nc"]

    input = np.ascontiguousarray(np.asarray(input), dtype=np.float32)
    target = np.ascontiguousarray(np.asarray(target), dtype=np.int32)

    in_maps = [
        {
            "input": input[c * NSHARD : (c + 1) * NSHARD],
            "target": target[c * NSHARD : (c + 1) * NSHARD],
        }
        for c in range(NCORES)
    ]
    res = run_bass_kernel_spmd(nc, in_maps, list(range(NCORES)))
    _cache["last_results"] = res

    total = 0.0
    for r in res.results:
        total += r["out"].astype(np.float64).sum()
    return np.asarray(-(total / N), dtype=np.float32)


# revision 34
# speedup vs baseline: 1.2053x; 1.2053x over previous
"""Balanced CE loss on 8 Trainium2 NeuronCores — raw Bass (hand-synced).

Math: z = t ? p*p : (1-p); loss = -mean(ln z)   (ln(p^2) == 2 ln p, w1=2, w0=1)

Engine split per chunk i:
  Sync  : dma p/t groups (see DMA_PLAN) -> +16 pl/tl lane sems
  ACT   : first p-group dma on its own HWDGE ring (parallel descriptor gen
          with Sync at startup), then LN_i ln(z) + accum col i -> +1 s_ln
  DVE   : Z_i   z = t ? p*p : 1-p   -> +1 s_z      (one fused custom DVE op,
          registered in dve_ops.OPS at import — the documented per-NEFF
          custom-op path; select cond is the raw int32 t, converted at the
          DVE read port)

With the select fused into one DVE instruction the per-chunk engine cost is
DVE ~2.3us + ACT ~2.3us against a ~4.9us DMA slot (2 MiB @ ~420 GB/s), so
the kernel is DMA-bound with ~2x compute slack instead of the knife-edge
balance of the 4-op version (SQ+LN on ACT, OM+CP on DVE ~= 5.0/4.6us/slot).
Measured fast-path structure: ~6.9us NEFF preamble (fixed rendezvous +
table DMAs) + ~0.8us HWDGE start + ~79.9us gapless input stream at
~420 GB/s + ~5us tail (last-chunk Z+LN+RA, store receipt, postamble).

DMA completion uses round-robin lane sems (a single sem cannot prove a given
tile landed when several DMAs are in flight: their 16 per-engine increments
interleave).  NRT does not reset semaphores between invocations, so ours are
cleared at the END of the kernel (past the Block-exit barrier); each run then
starts from zero with no start-of-kernel fence, letting the first DMA issue
as soon as the Sync engine boots.  The tail chunks taper so the
post-last-DMA compute chain is short.
"""

import numpy as np

import concourse.bacc as bacc
import concourse.bass as bass
import concourse.dve_ops as dve_ops
import concourse.mybir as mybir
from concourse.bass_utils import run_bass_kernel_spmd
from concourse.dve_spec import Spec, Src0, Src1, One, select, sq, lower, _has_src1
from concourse.dve_uop import DveOpSpec

N = 33554432
NCORES = 8
NSHARD = N // NCORES  # 4194304
P = 128
M = NSHARD // P  # 32768 f32 per partition

F = 2048  # slot width (max chunk width)
# Gradual tail taper: each tail chunk's Z+LN+READ_ACC must finish inside the
# stream time of the chunks after it (~2.47ns per unit width for the p+t
# pair at ~420 GB/s), or ACT work piles up serially after the stream ends.
# Searched numerically against the measured cost model; only the last
# chunk's Z+LN+RA+store chain stays exposed.
CHUNKS = [2048] * 14 + [1408, 1024, 768, 512, 384]
assert sum(CHUNKS) == M
NT = len(CHUNKS)

# DMA plan per stream: 2-chunk (2 MiB) groups at the head (compute unlocks
# early enough to pace slot recycling), 4-chunk (4 MiB) groups mid-stream
# (fewer transfer boundaries, closer to the large-transfer rate), 2-chunk to
# land before the taper. The tail chunks stay one DMA each so their
# completions arrive spread out for the taper overlap.
DMA_PLAN = [(0, 2), (2, 2), (4, 4), (8, 4), (12, 2)] + [
    (i, 1) for i in range(14, NT)
]

# One compute group per chunk. Paired (4096-wide) Z/LN groups were tried —
# they halve per-op fixed costs and help the DVFS-throttled mode — but a
# paired op near the taper is a ~4.3+4.0us serial block whose late start
# (gated by its merged DMA's completion) displaces the tail LNs past the
# stream end, and the fast-path samples regressed ~0.7us. Individual ops
# measured best (92.0us).
COMPUTE_PLAN = [(i, 1) for i in range(NT)]
NG = len(COMPUTE_PLAN)  # one acc column (partial sum) per compute group

_DMA_OF = {}
for _d, (_a, _n) in enumerate(DMA_PLAN):
    for _i in range(_a, _a + _n):
        _DMA_OF[_i] = _d


def dma_of(i):  # index of the DMA (per stream) that carries chunk i
    return _DMA_OF[i]


KP = 8  # p-tile slots (8 so 2-chunk DMAs at even chunk indices never wrap)
KT = 8  # t-tile slots
KZ = 4  # z slots
# 8 completion lanes per stream: a DMA's issue only waits for the DMA 8 back
# on its lane (one outstanding use per lane keeps the 16-inc count proof
# exact). With 4 lanes the ~600ns HWDGE issues couldn't run ahead through
# the 12 small tail DMAs and the SDMA engines idled ~2.4us mid-stream.
NL = 8

# merged DMAs land in consecutive F-wide slots, so they may only cover
# full-width chunks that don't wrap the slot rings
for _a, _n in DMA_PLAN:
    if _n > 1:
        assert (_a % KP) + _n <= KP and (_a % KT) + _n <= KT
        assert all(c == F for c in CHUNKS[_a : _a + _n])

WEIGHT0 = 1.0
WEIGHT1 = 2.0

_cache = {}

AF = mybir.ActivationFunctionType
ALU = mybir.AluOpType


def _register_bce_z():
    """z = select(t, p*p, 1-p) as one DVE instruction. Appended to
    dve_ops.OPS (the documented custom-op extension point); the uops sha is
    computed from lower() at import so the pin is self-consistent."""
    name = "BCE_Z_ANT"
    for op in dve_ops.OPS:
        if op.name == name:
            return op
    spec = Spec(
        body=select(Src1, sq(Src0), One - Src0),
        reference=lambda in0, in1: np.where(in1 != 0, in0 * in0, 1.0 - in0),
    )
    row = dve_ops._CUSTOM_DVE_ROW_BASE + len(dve_ops.OPS)
    shas = {
        ver: DveOpSpec(
            name=name, opcode=row, uops=lower(spec, ver=ver), rd1_en=_has_src1(spec)
        ).sha(ver)
        for ver in ("v3", "v4")
    }
    op = dve_ops.DveOp(name, spec, subdim=False, uops_sha=shas)
    dve_ops.OPS.append(op)
    dve_ops.CUSTOM_DVE_SPECS[name] = spec
    dve_ops._SUB_OPCODE_FOR_NAME[name] = row
    return op


BCE_Z = _register_bce_z()


def build_nc():
    # Bass.__init__ ends with an all_engine_barrier after the const memsets.
    # That barrier makes every engine wait for the slowest-booting one (~3.4us
    # measured, dominated by the unused PE/Tensor engine) before any DMA can
    # issue. Skip it; the only ordering it provided that this kernel needs is
    # const-memsets (GpSimd) vs ACT's bias read, covered by the s_const
    # handshake below.
    _orig_barrier = bass.Bass.all_engine_barrier
    bass.Bass.all_engine_barrier = lambda self, *a, **k: None
    try:
        nc = bacc.Bacc(
            "TRN2", target_bir_lowering=False, debug=False, num_devices=NCORES
        )
    finally:
        bass.Bass.all_engine_barrier = _orig_barrier

    x = nc.dram_tensor("input", [NSHARD], mybir.dt.float32, kind="ExternalInput").ap()
    t = nc.dram_tensor("target", [NSHARD], mybir.dt.int32, kind="ExternalInput").ap()
    out = nc.dram_tensor("out", [P, NG], mybir.dt.float32, kind="ExternalOutput").ap()

    xt = x.rearrange("(p m) -> p m", p=P)
    tt = t.rearrange("(p m) -> p m", p=P)

    offs = []
    o = 0
    for w in CHUNKS:
        offs.append(o)
        o += w

    pl = [nc.alloc_semaphore(f"s_p{j}") for j in range(NL)]
    tl = [nc.alloc_semaphore(f"s_t{j}") for j in range(NL)]
    s_out = nc.alloc_semaphore("s_out")
    s_z = nc.alloc_semaphore("s_z")
    s_ln = nc.alloc_semaphore("s_ln")
    s_const = nc.alloc_semaphore("s_const")
    sems = pl + tl + [s_out, s_z, s_ln, s_const]

    # Sems start at 0: zeroed by NRT at model load, and re-zeroed by OUR
    # end-of-kernel clears (after the Block-exit barrier) on every run. So no
    # start-of-kernel fence is needed and the first DMA can issue as soon as
    # the Sync engine boots. The only start-time ordering needed is the
    # framework's const memsets (GpSimd) vs ACT's bias read: a one-sem
    # handshake below covers it.

    with (
        nc.sbuf_tensor([P, KP * F], mybir.dt.float32) as pbuf,
        nc.sbuf_tensor([P, KT * F], mybir.dt.int32) as tbuf,
        nc.sbuf_tensor([P, KZ * F], mybir.dt.float32) as zbuf,
        nc.sbuf_tensor([P, NG], mybir.dt.float32) as acc,
        nc.sbuf_tensor([P, 1], mybir.dt.float32) as dummy,
        nc.psum_tensor([P, 2 * F], mybir.dt.float32) as lnall,
        nc.Block(no_gpsimd_drain=True) as block,
    ):

        # GpSimd ran the framework's const memsets in its preamble; publish
        # their completion for ACT (which reads the const-0.0 bias AP).
        @block.gpsimd
        def _(gp):
            gp.memset(dummy[:, :], 0.0).then_inc(s_const)

        # ---- Sync: DMA issue, paced by slot-free sems --------------------
        # The first p-group goes out on the Scalar engine's own HWDGE ring
        # (see the scalar block) concurrently with Sync's first t-group —
        # the two descriptor generators run in parallel, starting the
        # stream ~0.7us earlier.
        @block.sync
        def _(sync):
            for d, (a, n) in enumerate(DMA_PLAN):
                b = a + n - 1  # last chunk this DMA carries
                wsum = sum(CHUNKS[a : b + 1])
                if d > 0:
                    if b >= KP:
                        sync.wait_ge(s_z, b - KP + 1)
                    if d >= NL:
                        sync.wait_ge(pl[d % NL], 16 * (d // NL))
                    sync.dma_start(
                        out=pbuf[:, (a % KP) * F : (a % KP) * F + wsum],
                        in_=xt[:, offs[a] : offs[a] + wsum],
                    ).then_inc(pl[d % NL], 16)
                if b >= KT:
                    sync.wait_ge(s_z, b - KT + 1)
                if d >= NL:
                    sync.wait_ge(tl[d % NL], 16 * (d // NL))
                sync.dma_start(
                    out=tbuf[:, (a % KT) * F : (a % KT) * F + wsum],
                    in_=tt[:, offs[a] : offs[a] + wsum],
                ).then_inc(tl[d % NL], 16)
            sync.wait_ge(s_ln, NT)
            sync.dma_start(out=out[:], in_=acc[:]).then_inc(s_out, 16)
            # No completion wait here: the end-of-kernel dma_reset over our
            # sem range (emitted after the Block-exit barrier) drains this
            # DMA, so its receipt overlaps the barrier instead of preceding
            # it.

        # ---- DVE: z = t ? p*p : 1-p (one fused op per compute group) -----
        # s_z / s_ln stay in CHUNK units (group ops inc by the group size),
        # so the Sync slot-pacing waits are unchanged.
        @block.vector
        def _(vector):
            for a, n in COMPUTE_PLAN:
                b = a + n - 1
                w = sum(CHUNKS[a : b + 1])
                if b >= KZ:
                    vector.wait_ge(s_ln, b - KZ + 1)
                for d in sorted({dma_of(i) for i in range(a, b + 1)}):
                    vector.wait_ge(pl[d % NL], 16 * (d // NL + 1))
                    vector.wait_ge(tl[d % NL], 16 * (d // NL + 1))
                vector._custom_dve(
                    BCE_Z,
                    out=zbuf[:, (a % KZ) * F : (a % KZ) * F + w],
                    in0=pbuf[:, (a % KP) * F : (a % KP) * F + w],
                    in1=tbuf[:, (a % KT) * F : (a % KT) * F + w],
                ).then_inc(s_z, n)

        # ---- ACT: first p-group DMA on its own HWDGE ring, then ln(z)
        # with accum into acc column g (PSUM dump is same-engine in-order,
        # so the single lnall buffer needs no WAW sems) ---------------------
        @block.scalar
        def _(scalar):
            a0, n0 = DMA_PLAN[0]
            w0 = sum(CHUNKS[a0 : a0 + n0])
            scalar.dma_start(
                out=pbuf[:, (a0 % KP) * F : (a0 % KP) * F + w0],
                in_=xt[:, offs[a0] : offs[a0] + w0],
            ).then_inc(pl[0], 16)
            scalar.wait_ge(s_const, 1)
            for g, (a, n) in enumerate(COMPUTE_PLAN):
                b = a + n - 1
                w = sum(CHUNKS[a : b + 1])
                scalar.wait_ge(s_z, b + 1)
                scalar.activation(
                    lnall[:, :w],
                    zbuf[:, (a % KZ) * F : (a % KZ) * F + w],
                    AF.Ln,
                    accum_out=acc[:, g : g + 1],
                ).then_inc(s_ln, n)

    # Past the Block-exit barrier every engine is done: reset our sems (and
    # the DMA state tied to them, draining the in-flight output store) so the
    # next invocation starts from zero.
    for r in bass.compact_to_ranges([s.num for s in sems]):
        nc.gpsimd.dma_reset(r)
        nc.gpsimd.sem_clear(r)

    nc.compile()
    return nc


def kernel(input, target):
    if "nc" not in _cache:
        _cache["nc"] = build_nc()
    nc = _cache["nc"]

    input = np.ascontiguousarray(np.asarray(input), dtype=np.float32)
    target = np.ascontiguousarray(np.asarray(target), dtype=np.int32)

    in_maps = [
        {
            "input": input[c * NSHARD : (c + 1) * NSHARD],
            "target": target[c * NSHARD : (c + 1) * NSHARD],
        }
        for c in range(NCORES)
    ]
    res = run_bass_kernel_spmd(nc, in_maps, list(range(NCORES)))
    _cache["last_results"] = res

    total = 0.0
    for r in res.results:
        total += r["out"].astype(np.float64).sum()
    return np.asarray(-(total / N), dtype=np.float32)
